# revision 1
# baseline (speedup 1.0000x reference)
"""DKEPooling Trainium2 kernel.

Per-graph pipeline (d=256, n=512 nodes/graph):
  f = feat + 0.01*noise
  C' = f^T f - colsum(f)^T colssum(f)/n          (= (n-1)*cov, Gram + rank-1 PSUM trick)
  A  = C'/tr(C')
  Newton-Schulz (5 iter) reformulated via the commuting-polynomial invariant
  T_k := A Z_k^2:  T_{k+1} = 0.25 T_k (3I - T_k)^2   -> only 6 d^3 matmuls/graph
  (A^2, then 2 per T-step), followed by an 8-matvec tail applied to the mean
  (all remaining NS factors are applied vector-wise, never materialized).

Sharding: data-parallel over graphs. 8 cores x 16 graphs; no cross-core comm.
"""
import numpy as np

import concourse.bacc as bacc
import concourse.bass as bass
import concourse.mybir as mybir
import concourse.tile as tile
from concourse.bass_utils import run_bass_kernel_spmd

F32 = mybir.dt.float32
BF16 = mybir.dt.bfloat16
F32R = mybir.dt.float32r
ALU = mybir.AluOpType
ACTF = mybir.ActivationFunctionType

N_CORES = 8
D = 256
NPG = 512
B_TOTAL = 128
B_CORE = B_TOTAL // N_CORES      # 16 graphs per core
ROWS_CORE = B_CORE * NPG         # 8192 feat rows per core
W = 4                            # graphs per tail wave
N_WAVES = B_CORE // W

# const tensor layout (f32 [128, 772]):
#   [:, 0:256]   = [3I | 0]   (3I block for row-chunk 0)
#   [:, 256:512] = [0 | 3I]   (3I block for row-chunk 1)
#   [:, 512:640] = I128
#   [:, 640]     = ones column
#   [0, 641:769] = ones row
CST_COLS = 772


def _const_arrays():
    import ml_dtypes
    cst = np.zeros((128, CST_COLS), np.float32)
    eye = np.eye(128, dtype=np.float32)
    cst[:, 0:128] = 3.0 * eye
    cst[:, 384:512] = 3.0 * eye
    cst[:, 512:640] = eye
    cst[:, 640] = 1.0
    cst[0, 641:769] = 1.0
    cstb = np.ones((128, 1), ml_dtypes.bfloat16)
    cstr = np.eye(W, dtype=np.float32)
    return cst, cstb, cstr


def _r(ap):
    return ap.bitcast(F32R)


def build_module():
    nc = bacc.Bacc(None, target_bir_lowering=False)
    feat_d = nc.declare_dram_parameter("feat", [ROWS_CORE, D], F32, isOutput=False)
    noise_d = nc.declare_dram_parameter("noise", [ROWS_CORE, D], F32, isOutput=False)
    cst_d = nc.declare_dram_parameter("cst", [128, CST_COLS], F32, isOutput=False)
    cstb_d = nc.declare_dram_parameter("cstb", [128, 1], BF16, isOutput=False)
    cstr_d = nc.declare_dram_parameter("cstr", [W, W], F32R, isOutput=False)
    out_d = nc.declare_dram_parameter("out", [B_CORE, D], F32, isOutput=True)

    with tile.TileContext(nc) as tc:
        _build_tile(tc, nc, feat_d, noise_d, cst_d, cstb_d, cstr_d, out_d)
    nc.compile()
    return nc


def _build_tile(tc, nc, feat_d, noise_d, cst_d, cstb_d, cstr_d, out_d):
    import contextlib
    ctx = contextlib.ExitStack()
    with ctx:
        stage_p = ctx.enter_context(tc.tile_pool(name="stage", bufs=5))
        g_p = ctx.enter_context(tc.tile_pool(name="gp", bufs=6))
        mats_p = ctx.enter_context(tc.tile_pool(name="mats", bufs=7))
        chain_p = ctx.enter_context(tc.tile_pool(name="chain", bufs=3))
        small_p = ctx.enter_context(tc.tile_pool(name="small", bufs=6))
        rows_p = ctx.enter_context(tc.tile_pool(name="rows", bufs=3))
        tail_p = ctx.enter_context(tc.tile_pool(name="tailp", bufs=3))
        cst_p = ctx.enter_context(tc.tile_pool(name="cstp", bufs=1))
        psG = ctx.enter_context(tc.tile_pool(name="psG", bufs=3, space="PSUM"))
        psS = ctx.enter_context(tc.tile_pool(name="psS", bufs=1, space="PSUM"))
        psUR = ctx.enter_context(tc.tile_pool(name="psUR", bufs=2, space="PSUM"))
        psT = ctx.enter_context(tc.tile_pool(name="psT", bufs=2, space="PSUM"))

        cst = cst_p.tile([128, CST_COLS], F32, tag="cst", name="cst_sb")
        nc.gpsimd.dma_start(cst, cst_d[:, :])
        onesb = cst_p.tile([128, 1], BF16, tag="onesb", name="onesb_sb")
        nc.gpsimd.dma_start(onesb, cstb_d[:, :])
        IWr = cst_p.tile([W, W], F32R, tag="iwr", name="iwr_sb")
        nc.gpsimd.dma_start(IWr, cstr_d[:, :])

        def c3I(m):
            return cst[:, 256 * m:256 * (m + 1)]

        I128 = cst[:, 512:640]
        ones_col = cst[:, 640:641]
        ones_row = cst[0:1, 641:769]

        def phase_a(g, V0ROWS, b):
            """Load graph g, compute A and T1..T3; returns dict of kept tiles."""
            # One big [128, 4*256] tile per tensor: the graph's 512 rows as 4
            # row-chunks side by side in the free dim; a single SWDGE DMA each
            # (HWDGE fans one transfer across several queue semaphores, which
            # overflows the DVE consumers' wait slots).
            ft = stage_p.tile([128, 4 * D], F32, tag="ft", name=f"ft_{g}")
            nc.gpsimd.dma_start(
                ft, feat_d[g * NPG:(g + 1) * NPG, :].rearrange("(c p) d -> p c d", p=128))
            nz = stage_p.tile([128, 4 * D], F32, tag="nz", name=f"nz_{g}")
            nc.gpsimd.dma_start(
                nz, noise_d[g * NPG:(g + 1) * NPG, :].rearrange("(c p) d -> p c d", p=128))
            gb = g_p.tile([128, 4 * D], BF16, tag="g", name=f"g_{g}")
            # f = (noise * 0.01) + feat, rounded to bf16 for the Gram.
            # Keep the DVE arithmetic in plain f32 (in-place, standard ISA
            # structs with enough sync slots) and convert to bf16 on ACT;
            # dtype-converting DVE ops lower to custom ucode with too few
            # sync-wait slots for walrus.
            nc.vector.scalar_tensor_tensor(gb, nz, 0.01, ft, ALU.mult, ALU.add)
            gt = [gb[:, k * D:(k + 1) * D] for k in range(4)]

            # Gram into PSUM: G_m = sum_k g_k[:, m*128:...].T @ g_k   (stop on corr MM)
            G = [psG.tile([128, D], F32, tag="G", name=f"G{m}_{g}") for m in range(2)]
            for k in range(4):
                for m in range(2):
                    nc.tensor.matmul(G[m], gt[k][:, m * 128:(m + 1) * 128], gt[k],
                                     start=(k == 0), stop=False)
            # column sums s = ones^T g
            s_ps = psS.tile([1, D], F32, tag="ps_small", name=f"s_{g}")
            for k in range(4):
                nc.tensor.matmul(s_ps, onesb, gt[k], start=(k == 0), stop=(k == 3))
            srow = small_p.tile([1, D], BF16, tag="srow", name=f"srow_{g}")
            nc.scalar.copy(srow, s_ps)
            srow_n = small_p.tile([1, D], BF16, tag="srow_n", name=f"srown_{g}")
            nc.vector.tensor_scalar_mul(srow_n, srow, -1.0 / NPG)

            # rank-1 mean correction accumulated into the Gram PSUM:
            # C' = G - s^T s / n
            for m in range(2):
                nc.tensor.matmul(G[m], srow_n[0:1, m * 128:(m + 1) * 128], srow,
                                 start=False, stop=True)

            # Evacuate C' from PSUM via ACT (DVE reads of PSUM crash the
            # exec unit on this runtime; ACT reads are fine).
            Gc = []
            for m in range(2):
                gc = chain_p.tile([128, D], F32, tag=f"Gc{m}", name=f"Gc{m}_{g}")
                nc.scalar.copy(gc, G[m])
                Gc.append(gc)
            # trace via diag mask + GPSIMD partition all-reduce (the tiny
            # fp32 PE matmuls this used before crash the exec unit)
            scr = small_p.tile([128, 128], F32, tag="scr", name=f"scr_{g}")
            dg = small_p.tile([128, 2], F32, tag="dg", name=f"dg_{g}")
            for m in range(2):
                nc.vector.scalar_tensor_tensor(scr, Gc[m][:, m * 128:(m + 1) * 128],
                                               1.0, I128, ALU.mult, ALU.mult,
                                               accum_out=dg[:, m:m + 1])
            import concourse.bass_isa as bass_isa
            dgs = small_p.tile([128, 1], F32, tag="dgs", name=f"dgs_{g}")
            nc.vector.tensor_add(dgs, dg[:, 0:1], dg[:, 1:2])
            trc = small_p.tile([128, 1], F32, tag="trc", name=f"trc_{g}")
            nc.gpsimd.partition_all_reduce(trc, dgs, 128, bass_isa.ReduceOp.add)
            rcpb = small_p.tile([128, 1], F32, tag="rcpb", name=f"rcpb_{g}")
            nc.vector.reciprocal(rcpb, trc)
            sq = small_p.tile([1, 1], F32, tag="sq", name=f"sq_{g}")
            nc.scalar.activation(sq, trc[0:1, 0:1], ACTF.Sqrt, scale=1.0 / (NPG - 1))
            cb = small_p.tile([1, 1], F32, tag="cb", name=f"cb_{g}")
            nc.vector.tensor_scalar_mul(cb, sq, 0.03125 / NPG)
            # v0 row for the tail: mean scaled by all folded constants.
            # Computed at partition 0, DMA'd into row b of V0ROWS (compute
            # engines cannot write non-32-aligned partition bases).
            v0r = small_p.tile([1, D], F32R, tag="v0r", name=f"v0r_{g}")
            nc.scalar.activation(v0r, s_ps, ACTF.Copy, scale=cb)
            nc.sync.dma_start(V0ROWS[b:b + 1, :], v0r)

            A = []
            for m in range(2):
                Am = mats_p.tile([128, D], F32R, tag=f"A{m}", name=f"A{m}_{g}")
                nc.vector.tensor_scalar_mul(Am, Gc[m], rcpb)
                A.append(Am)

            def mm256(tag, L, R, dst_pool, dst_tag):
                dst = [dst_pool.tile([128, D], F32, tag=dst_tag, name=f"{tag}{m}_{g}")
                       for m in range(2)]
                for m in range(2):
                    for k in range(2):
                        nc.tensor.matmul(dst[m], L[k][:, m * 128:(m + 1) * 128],
                                         R[k], start=(k == 0), stop=(k == 1))
                return dst

            # T-chain: A2 -> T1 -> T2 -> T3 (2 matmuls per step after A2)
            A2 = mm256("A2", A, A, psG, "G")
            W1 = []
            V0 = []
            for m in range(2):
                a2c = chain_p.tile([128, D], F32, tag=f"A2c{m}", name=f"A2c{m}_{g}")
                nc.scalar.copy(a2c, A2[m])
                w1 = chain_p.tile([128, D], F32R, tag=f"W1{m}", name=f"W1{m}_{g}")
                nc.vector.scalar_tensor_tensor(w1, A[m], 3.0, a2c, ALU.mult, ALU.subtract)
                W1.append(w1)
                v0 = chain_p.tile([128, D], F32R, tag=f"V0{m}", name=f"V0{m}_{g}")
                nc.vector.scalar_tensor_tensor(v0, A[m], -1.0, c3I(m), ALU.mult, ALU.add)
                V0.append(v0)
            P = mm256("P", W1, V0, psG, "G")
            T1 = []
            V1 = []
            for m in range(2):
                t1 = mats_p.tile([128, D], F32R, tag=f"T1{m}", name=f"T1{m}_{g}")
                nc.scalar.mul(t1, P[m], 0.25)
                T1.append(t1)
                v1 = chain_p.tile([128, D], F32R, tag=f"V1{m}", name=f"V1{m}_{g}")
                nc.vector.scalar_tensor_tensor(v1, t1, -1.0, c3I(m), ALU.mult, ALU.add)
                V1.append(v1)
            Q = mm256("Q", T1, V1, psG, "G")
            Qb = []
            for m in range(2):
                qb = chain_p.tile([128, D], F32R, tag=f"Qb{m}", name=f"Qb{m}_{g}")
                nc.scalar.copy(qb, Q[m])
                Qb.append(qb)
            R = mm256("R", Qb, V1, psG, "G")
            T2 = []
            V2 = []
            for m in range(2):
                t2 = mats_p.tile([128, D], F32R, tag=f"T2{m}", name=f"T2{m}_{g}")
                nc.scalar.mul(t2, R[m], 0.25)
                T2.append(t2)
                v2 = chain_p.tile([128, D], F32R, tag=f"V2{m}", name=f"V2{m}_{g}")
                nc.vector.scalar_tensor_tensor(v2, t2, -1.0, c3I(m), ALU.mult, ALU.add)
                V2.append(v2)
            S = mm256("S", T2, V2, psG, "G")
            Sb = []
            for m in range(2):
                sb_ = chain_p.tile([128, D], F32R, tag=f"Sb{m}", name=f"Sb{m}_{g}")
                nc.scalar.copy(sb_, S[m])
                Sb.append(sb_)
            U = mm256("U", Sb, V2, psG, "G")
            T3 = []
            for m in range(2):
                t3 = mats_p.tile([128, D], F32R, tag=f"T3{m}", name=f"T3{m}_{g}")
                nc.scalar.mul(t3, U[m], 0.25)
                T3.append(t3)
            return {"A": A, "T1": T1, "T2": T2, "T3": T3}

        def matvec_step(si, wave, cur, mats, kind, v0c=None):
            """One tail step for all W graphs: u = X @ v (row-form + transpose back).

            Per-graph u rows land in PSUM at 32-aligned partitions (legal PE
            column-group bases), then a strided DMA gathers them to packed rows.
            Returns next v column tiles [128, W] x2."""
            xkey = {0: "T3", 1: "T3", 2: "T3", 3: "T3", 4: "T2", 5: "T1", 6: "A", 7: "A"}[si]
            usb = rows_p.tile([W, D], F32R, tag="usb", name=f"usb_{wave}_{si}")
            for b in range(W):
                X = mats[b][xkey]
                ur = psUR.tile([1, D], F32, tag="ur", name=f"ur_{wave}_{si}_{b}")
                for k in range(2):
                    nc.tensor.matmul(ur, cur[k][:, b:b + 1], X[k],
                                     start=(k == 0), stop=(k == 1))
                # PE can only write PSUM at base partition 0 here, and compute
                # engines cannot write partition b directly: copy to a
                # partition-0 row, then DMA-scatter into the packed row tile.
                us = small_p.tile([1, D], F32R, tag="us", name=f"us_{wave}_{si}_{b}")
                nc.scalar.copy(us, ur)
                if kind == "final":
                    nc.sync.dma_start(out_d[wave * W + b: wave * W + b + 1, :], us.bitcast(F32))
                else:
                    nc.sync.dma_start(usb[b:b + 1, :], us)
            if kind == "final":
                return None
            uc = psT.tile([128, 2 * W], F32, tag="ucols", name=f"uc_{wave}_{si}")
            for m in range(2):
                nc.tensor.matmul(uc[:, m * W:(m + 1) * W],
                                 usb[:, m * 128:(m + 1) * 128], IWr)
            nxt = [tail_p.tile([128, W], F32R, tag=f"VC{m}", name=f"vc_{wave}_{si}_{m}")
                   for m in range(2)]
            for m in range(2):
                ucm = uc[:, m * W:(m + 1) * W]
                if kind == "comb":
                    ucs = tail_p.tile([128, W], F32, tag=f"ucs{m}", name=f"ucs_{wave}_{si}_{m}")
                    nc.scalar.copy(ucs, ucm)
                    nc.vector.scalar_tensor_tensor(nxt[m], cur[m], 3.0, ucs,
                                                   ALU.mult, ALU.subtract)
                elif kind == "a3":
                    # v4 = 3*v0 - 0.25*u
                    a3q = tail_p.tile([128, W], F32R, tag=f"a3q{m}", name=f"a3q_{wave}_{m}")
                    nc.scalar.mul(a3q, ucm, 0.25)
                    nc.vector.scalar_tensor_tensor(nxt[m], v0c[m], 3.0, a3q,
                                                   ALU.mult, ALU.subtract)
            return nxt

        for wave in range(N_WAVES):
            V0ROWS = rows_p.tile([W, D], F32R, tag="v0rows", name=f"v0rows_{wave}")
            mats = []
            for b in range(W):
                g = wave * W + b
                mats.append(phase_a(g, V0ROWS, b))

            # transpose v0 rows -> column tiles [128, W] x2
            v0ps = psT.tile([128, 2 * W], F32, tag="ucols", name=f"v0ps_{wave}")
            for m in range(2):
                nc.tensor.matmul(v0ps[:, m * W:(m + 1) * W],
                                 V0ROWS[:, m * 128:(m + 1) * 128], IWr)
            v0c = []
            for m in range(2):
                v = tail_p.tile([128, W], F32R, tag=f"VC{m}", name=f"v0c_{wave}_{m}")
                nc.scalar.copy(v, v0ps[:, m * W:(m + 1) * W])
                v0c.append(v)

            cur = v0c
            kinds = ["comb", "comb", "a3", "comb", "comb", "comb", "comb", "final"]
            for si in range(8):
                cur = matvec_step(si, wave, cur, mats, kinds[si],
                                  v0c=v0c if kinds[si] == "a3" else None)


_CACHED_NC = None


def _get_nc():
    global _CACHED_NC
    if _CACHED_NC is None:
        _CACHED_NC = build_module()
    return _CACHED_NC


def _run(feat, noise, **spmd_kwargs):
    feat = np.ascontiguousarray(np.asarray(feat), dtype=np.float32)
    noise = np.ascontiguousarray(np.asarray(noise), dtype=np.float32)
    cst, cstb, cstr = _const_arrays()
    nc = _get_nc()
    in_maps = []
    for c in range(N_CORES):
        in_maps.append({
            "feat": feat[c * ROWS_CORE:(c + 1) * ROWS_CORE],
            "noise": noise[c * ROWS_CORE:(c + 1) * ROWS_CORE],
            "cst": cst,
            "cstb": cstb,
            "cstr": cstr,
        })
    return run_bass_kernel_spmd(nc, in_maps, list(range(N_CORES)), **spmd_kwargs)


def kernel(feat, noise, n_per_graph):
    assert int(n_per_graph) == NPG
    try:
        res = _run(feat, noise)
    except Exception:
        # the axon device occasionally reports a transient unrecoverable
        # state; one retry usually succeeds
        res = _run(feat, noise)
    return np.concatenate([res.results[c]["out"] for c in range(N_CORES)], axis=0)



# revision 5
# speedup vs baseline: 35.8480x; 35.8480x over previous
"""DKEPooling Trainium2 kernel.

Per-graph pipeline (d=256, n=512 nodes/graph):
  f = feat + 0.01*noise
  C' = f^T f - colsum(f)^T colssum(f)/n          (= (n-1)*cov, Gram + rank-1 PSUM trick)
  A  = C'/tr(C')
  Newton-Schulz (5 iter) reformulated via the commuting-polynomial invariant
  T_k := A Z_k^2:  T_{k+1} = 0.25 T_k (3I - T_k)^2   -> only 6 d^3 matmuls/graph
  (A^2, then 2 per T-step), followed by an 8-matvec tail applied to the mean
  (all remaining NS factors are applied vector-wise, never materialized).

Sharding: data-parallel over graphs. 8 cores x 16 graphs; no cross-core comm.
"""
import numpy as np

import concourse.bacc as bacc
import concourse.bass as bass
import concourse.mybir as mybir
import concourse.tile as tile
from concourse.bass_utils import run_bass_kernel_spmd

F32 = mybir.dt.float32
BF16 = mybir.dt.bfloat16
F32R = mybir.dt.float32r
ALU = mybir.AluOpType
ACTF = mybir.ActivationFunctionType

N_CORES = 8
D = 256
NPG = 512
B_TOTAL = 128
B_CORE = B_TOTAL // N_CORES      # 16 graphs per core
ROWS_CORE = B_CORE * NPG         # 8192 feat rows per core
W = 4                            # graphs per tail wave
N_WAVES = B_CORE // W

# const tensor layout (f32 [128, 772]):
#   [:, 0:256]   = [3I | 0]   (3I block for row-chunk 0)
#   [:, 256:512] = [0 | 3I]   (3I block for row-chunk 1)
#   [:, 512:640] = I128
#   [:, 640]     = ones column
#   [0, 641:769] = ones row
CST_COLS = 772


def _const_arrays():
    import ml_dtypes
    cst = np.zeros((128, CST_COLS), np.float32)
    eye = np.eye(128, dtype=np.float32)
    cst[:, 0:128] = 3.0 * eye
    cst[:, 384:512] = 3.0 * eye
    cst[:, 512:640] = eye
    cst[:, 640] = 1.0
    cst[0, 641:769] = 1.0
    cstb = np.ones((128, 1), ml_dtypes.bfloat16)
    cstr = np.eye(W, dtype=np.float32)
    return cst, cstb, cstr


def _r(ap):
    return ap.bitcast(F32R)


def build_module(reps=1):
    """reps>1 unrolls the whole body N times in one NEFF (same inputs and
    outputs each rep) — used only by test.py's slope-based device timing."""
    nc = bacc.Bacc(None, target_bir_lowering=False)
    feat_d = nc.declare_dram_parameter("feat", [ROWS_CORE, D], F32, isOutput=False)
    noise_d = nc.declare_dram_parameter("noise", [ROWS_CORE, D], F32, isOutput=False)
    cst_d = nc.declare_dram_parameter("cst", [128, CST_COLS], F32, isOutput=False)
    cstb_d = nc.declare_dram_parameter("cstb", [128, 1], BF16, isOutput=False)
    cstr_d = nc.declare_dram_parameter("cstr", [W, W], F32R, isOutput=False)
    out_d = nc.declare_dram_parameter("out", [B_CORE, D], F32, isOutput=True)

    with tile.TileContext(nc) as tc:
        _build_tile(tc, nc, feat_d, noise_d, cst_d, cstb_d, cstr_d, out_d,
                    reps=reps)
    nc.compile()
    return nc


def _build_tile(tc, nc, feat_d, noise_d, cst_d, cstb_d, cstr_d, out_d, reps=1):
    import contextlib
    ctx = contextlib.ExitStack()
    with ctx:
        stage_p = ctx.enter_context(tc.tile_pool(name="stage", bufs=5))
        g_p = ctx.enter_context(tc.tile_pool(name="gp", bufs=6))
        mats_p = ctx.enter_context(tc.tile_pool(name="mats", bufs=7))
        chain_p = ctx.enter_context(tc.tile_pool(name="chain", bufs=3))
        small_p = ctx.enter_context(tc.tile_pool(name="small", bufs=6))
        rows_p = ctx.enter_context(tc.tile_pool(name="rows", bufs=3))
        tail_p = ctx.enter_context(tc.tile_pool(name="tailp", bufs=3))
        cst_p = ctx.enter_context(tc.tile_pool(name="cstp", bufs=1))
        psG = ctx.enter_context(tc.tile_pool(name="psG", bufs=3, space="PSUM"))
        psS = ctx.enter_context(tc.tile_pool(name="psS", bufs=1, space="PSUM"))
        psUR = ctx.enter_context(tc.tile_pool(name="psUR", bufs=2, space="PSUM"))
        psT = ctx.enter_context(tc.tile_pool(name="psT", bufs=2, space="PSUM"))

        cst = cst_p.tile([128, CST_COLS], F32, tag="cst", name="cst_sb")
        nc.gpsimd.dma_start(cst, cst_d[:, :])
        onesb = cst_p.tile([128, 1], BF16, tag="onesb", name="onesb_sb")
        nc.gpsimd.dma_start(onesb, cstb_d[:, :])
        IWr = cst_p.tile([W, W], F32R, tag="iwr", name="iwr_sb")
        nc.gpsimd.dma_start(IWr, cstr_d[:, :])

        def c3I(m):
            return cst[:, 256 * m:256 * (m + 1)]

        I128 = cst[:, 512:640]
        ones_col = cst[:, 640:641]
        ones_row = cst[0:1, 641:769]

        def phase_a(g, V0ROWS, b):
            """Load graph g, compute A and T1..T3; returns dict of kept tiles."""
            # One big [128, 4*256] tile per tensor: the graph's 512 rows as 4
            # row-chunks side by side in the free dim; a single SWDGE DMA each
            # (HWDGE fans one transfer across several queue semaphores, which
            # overflows the DVE consumers' wait slots).
            ga = g % B_CORE
            ft = stage_p.tile([128, 4 * D], F32, tag="ft", name=f"ft_{g}")
            nc.gpsimd.dma_start(
                ft, feat_d[ga * NPG:(ga + 1) * NPG, :].rearrange("(c p) d -> p c d", p=128))
            nz = stage_p.tile([128, 4 * D], F32, tag="nz", name=f"nz_{g}")
            nc.gpsimd.dma_start(
                nz, noise_d[ga * NPG:(ga + 1) * NPG, :].rearrange("(c p) d -> p c d", p=128))
            gb = g_p.tile([128, 4 * D], BF16, tag="g", name=f"g_{g}")
            # f = (noise * 0.01) + feat, rounded to bf16 for the Gram.
            # Keep the DVE arithmetic in plain f32 (in-place, standard ISA
            # structs with enough sync slots) and convert to bf16 on ACT;
            # dtype-converting DVE ops lower to custom ucode with too few
            # sync-wait slots for walrus.
            nc.vector.scalar_tensor_tensor(gb, nz, 0.01, ft, ALU.mult, ALU.add)
            gt = [gb[:, k * D:(k + 1) * D] for k in range(4)]

            # Gram into PSUM: G_m = sum_k g_k[:, m*128:...].T @ g_k   (stop on corr MM)
            G = [psG.tile([128, D], F32, tag="G", name=f"G{m}_{g}") for m in range(2)]
            for k in range(4):
                for m in range(2):
                    nc.tensor.matmul(G[m], gt[k][:, m * 128:(m + 1) * 128], gt[k],
                                     start=(k == 0), stop=False)
            # column sums s = ones^T g
            s_ps = psS.tile([1, D], F32, tag="ps_small", name=f"s_{g}")
            for k in range(4):
                nc.tensor.matmul(s_ps, onesb, gt[k], start=(k == 0), stop=(k == 3))
            srow = small_p.tile([1, D], BF16, tag="srow", name=f"srow_{g}")
            nc.scalar.copy(srow, s_ps)
            srow_n = small_p.tile([1, D], BF16, tag="srow_n", name=f"srown_{g}")
            nc.vector.tensor_scalar_mul(srow_n, srow, -1.0 / NPG)

            # rank-1 mean correction accumulated into the Gram PSUM:
            # C' = G - s^T s / n
            for m in range(2):
                nc.tensor.matmul(G[m], srow_n[0:1, m * 128:(m + 1) * 128], srow,
                                 start=False, stop=True)

            # Evacuate C' from PSUM via ACT (DVE reads of PSUM crash the
            # exec unit on this runtime; ACT reads are fine).
            Gc = []
            for m in range(2):
                gc = chain_p.tile([128, D], F32, tag=f"Gc{m}", name=f"Gc{m}_{g}")
                nc.scalar.copy(gc, G[m])
                Gc.append(gc)
            # trace via diag mask + GPSIMD partition all-reduce (the tiny
            # fp32 PE matmuls this used before crash the exec unit)
            scr = small_p.tile([128, 128], F32, tag="scr", name=f"scr_{g}")
            dg = small_p.tile([128, 2], F32, tag="dg", name=f"dg_{g}")
            for m in range(2):
                nc.vector.scalar_tensor_tensor(scr, Gc[m][:, m * 128:(m + 1) * 128],
                                               1.0, I128, ALU.mult, ALU.mult,
                                               accum_out=dg[:, m:m + 1])
            import concourse.bass_isa as bass_isa
            dgs = small_p.tile([128, 1], F32, tag="dgs", name=f"dgs_{g}")
            nc.vector.tensor_add(dgs, dg[:, 0:1], dg[:, 1:2])
            trc = small_p.tile([128, 1], F32, tag="trc", name=f"trc_{g}")
            nc.gpsimd.partition_all_reduce(trc, dgs, 128, bass_isa.ReduceOp.add)
            rcpb = small_p.tile([128, 1], F32, tag="rcpb", name=f"rcpb_{g}")
            nc.vector.reciprocal(rcpb, trc)
            sq = small_p.tile([1, 1], F32, tag="sq", name=f"sq_{g}")
            nc.scalar.activation(sq, trc[0:1, 0:1], ACTF.Sqrt, scale=1.0 / (NPG - 1))
            cb = small_p.tile([1, 1], F32, tag="cb", name=f"cb_{g}")
            nc.vector.tensor_scalar_mul(cb, sq, 0.03125 / NPG)
            # v0 row for the tail: mean scaled by all folded constants.
            # Computed at partition 0, DMA'd into row b of V0ROWS (compute
            # engines cannot write non-32-aligned partition bases).
            v0r = small_p.tile([1, D], F32R, tag="v0r", name=f"v0r_{g}")
            nc.scalar.activation(v0r, s_ps, ACTF.Copy, scale=cb)
            nc.sync.dma_start(V0ROWS[b:b + 1, :], v0r)

            A = []
            for m in range(2):
                Am = mats_p.tile([128, D], F32R, tag=f"A{m}", name=f"A{m}_{g}")
                nc.vector.tensor_scalar_mul(Am, Gc[m], rcpb)
                A.append(Am)

            def mm256(tag, L, R, dst_pool, dst_tag):
                dst = [dst_pool.tile([128, D], F32, tag=dst_tag, name=f"{tag}{m}_{g}")
                       for m in range(2)]
                for m in range(2):
                    for k in range(2):
                        nc.tensor.matmul(dst[m], L[k][:, m * 128:(m + 1) * 128],
                                         R[k], start=(k == 0), stop=(k == 1))
                return dst

            # T-chain: A2 -> T1 -> T2 -> T3 (2 matmuls per step after A2)
            A2 = mm256("A2", A, A, psG, "G")
            W1 = []
            V0 = []
            for m in range(2):
                a2c = chain_p.tile([128, D], F32, tag=f"A2c{m}", name=f"A2c{m}_{g}")
                nc.scalar.copy(a2c, A2[m])
                w1 = chain_p.tile([128, D], F32R, tag=f"W1{m}", name=f"W1{m}_{g}")
                nc.vector.scalar_tensor_tensor(w1, A[m], 3.0, a2c, ALU.mult, ALU.subtract)
                W1.append(w1)
                v0 = chain_p.tile([128, D], F32R, tag=f"V0{m}", name=f"V0{m}_{g}")
                nc.vector.scalar_tensor_tensor(v0, A[m], -1.0, c3I(m), ALU.mult, ALU.add)
                V0.append(v0)
            P = mm256("P", W1, V0, psG, "G")
            T1 = []
            V1 = []
            for m in range(2):
                t1 = mats_p.tile([128, D], F32R, tag=f"T1{m}", name=f"T1{m}_{g}")
                nc.scalar.mul(t1, P[m], 0.25)
                T1.append(t1)
                v1 = chain_p.tile([128, D], F32R, tag=f"V1{m}", name=f"V1{m}_{g}")
                nc.vector.scalar_tensor_tensor(v1, t1, -1.0, c3I(m), ALU.mult, ALU.add)
                V1.append(v1)
            Q = mm256("Q", T1, V1, psG, "G")
            Qb = []
            for m in range(2):
                qb = chain_p.tile([128, D], F32R, tag=f"Qb{m}", name=f"Qb{m}_{g}")
                nc.scalar.copy(qb, Q[m])
                Qb.append(qb)
            R = mm256("R", Qb, V1, psG, "G")
            T2 = []
            V2 = []
            for m in range(2):
                t2 = mats_p.tile([128, D], F32R, tag=f"T2{m}", name=f"T2{m}_{g}")
                nc.scalar.mul(t2, R[m], 0.25)
                T2.append(t2)
                v2 = chain_p.tile([128, D], F32R, tag=f"V2{m}", name=f"V2{m}_{g}")
                nc.vector.scalar_tensor_tensor(v2, t2, -1.0, c3I(m), ALU.mult, ALU.add)
                V2.append(v2)
            S = mm256("S", T2, V2, psG, "G")
            Sb = []
            for m in range(2):
                sb_ = chain_p.tile([128, D], F32R, tag=f"Sb{m}", name=f"Sb{m}_{g}")
                nc.scalar.copy(sb_, S[m])
                Sb.append(sb_)
            U = mm256("U", Sb, V2, psG, "G")
            T3 = []
            for m in range(2):
                t3 = mats_p.tile([128, D], F32R, tag=f"T3{m}", name=f"T3{m}_{g}")
                nc.scalar.mul(t3, U[m], 0.25)
                T3.append(t3)
            return {"A": A, "T1": T1, "T2": T2, "T3": T3}

        def matvec_step(si, wave, cur, mats, kind, v0c=None):
            """One tail step for all W graphs: u = X @ v (row-form + transpose back).

            Per-graph u rows land in PSUM at 32-aligned partitions (legal PE
            column-group bases), then a strided DMA gathers them to packed rows.
            Returns next v column tiles [128, W] x2."""
            xkey = {0: "T3", 1: "T3", 2: "T3", 3: "T3", 4: "T2", 5: "T1", 6: "A", 7: "A"}[si]
            usb = rows_p.tile([W, D], F32R, tag="usb", name=f"usb_{wave}_{si}")
            for b in range(W):
                X = mats[b][xkey]
                ur = psUR.tile([1, D], F32, tag="ur", name=f"ur_{wave}_{si}_{b}")
                for k in range(2):
                    nc.tensor.matmul(ur, cur[k][:, b:b + 1], X[k],
                                     start=(k == 0), stop=(k == 1))
                # PE can only write PSUM at base partition 0 here, and compute
                # engines cannot write partition b directly: copy to a
                # partition-0 row, then DMA-scatter into the packed row tile.
                us = small_p.tile([1, D], F32R, tag="us", name=f"us_{wave}_{si}_{b}")
                nc.scalar.copy(us, ur)
                if kind == "final":
                    ob = (wave * W + b) % B_CORE
                    nc.sync.dma_start(out_d[ob: ob + 1, :], us.bitcast(F32))
                else:
                    nc.sync.dma_start(usb[b:b + 1, :], us)
            if kind == "final":
                return None
            uc = psT.tile([128, 2 * W], F32, tag="ucols", name=f"uc_{wave}_{si}")
            for m in range(2):
                nc.tensor.matmul(uc[:, m * W:(m + 1) * W],
                                 usb[:, m * 128:(m + 1) * 128], IWr)
            nxt = [tail_p.tile([128, W], F32R, tag=f"VC{m}", name=f"vc_{wave}_{si}_{m}")
                   for m in range(2)]
            for m in range(2):
                ucm = uc[:, m * W:(m + 1) * W]
                if kind == "comb":
                    ucs = tail_p.tile([128, W], F32, tag=f"ucs{m}", name=f"ucs_{wave}_{si}_{m}")
                    nc.scalar.copy(ucs, ucm)
                    nc.vector.scalar_tensor_tensor(nxt[m], cur[m], 3.0, ucs,
                                                   ALU.mult, ALU.subtract)
                elif kind == "a3":
                    # v4 = 3*v0 - 0.25*u
                    a3q = tail_p.tile([128, W], F32R, tag=f"a3q{m}", name=f"a3q_{wave}_{m}")
                    nc.scalar.mul(a3q, ucm, 0.25)
                    nc.vector.scalar_tensor_tensor(nxt[m], v0c[m], 3.0, a3q,
                                                   ALU.mult, ALU.subtract)
            return nxt

        for wave in range(N_WAVES * reps):
            V0ROWS = rows_p.tile([W, D], F32R, tag="v0rows", name=f"v0rows_{wave}")
            mats = []
            for b in range(W):
                g = wave * W + b
                mats.append(phase_a(g, V0ROWS, b))

            # transpose v0 rows -> column tiles [128, W] x2
            v0ps = psT.tile([128, 2 * W], F32, tag="ucols", name=f"v0ps_{wave}")
            for m in range(2):
                nc.tensor.matmul(v0ps[:, m * W:(m + 1) * W],
                                 V0ROWS[:, m * 128:(m + 1) * 128], IWr)
            v0c = []
            for m in range(2):
                v = tail_p.tile([128, W], F32R, tag=f"VC{m}", name=f"v0c_{wave}_{m}")
                nc.scalar.copy(v, v0ps[:, m * W:(m + 1) * W])
                v0c.append(v)

            cur = v0c
            kinds = ["comb", "comb", "a3", "comb", "comb", "comb", "comb", "final"]
            for si in range(8):
                cur = matvec_step(si, wave, cur, mats, kinds[si],
                                  v0c=v0c if kinds[si] == "a3" else None)


_CACHED_NC = None


def _get_nc():
    global _CACHED_NC
    if _CACHED_NC is None:
        _CACHED_NC = build_module()
    return _CACHED_NC


def _run(feat, noise, **spmd_kwargs):
    feat = np.ascontiguousarray(np.asarray(feat), dtype=np.float32)
    noise = np.ascontiguousarray(np.asarray(noise), dtype=np.float32)
    cst, cstb, cstr = _const_arrays()
    nc = _get_nc()
    in_maps = []
    for c in range(N_CORES):
        in_maps.append({
            "feat": feat[c * ROWS_CORE:(c + 1) * ROWS_CORE],
            "noise": noise[c * ROWS_CORE:(c + 1) * ROWS_CORE],
            "cst": cst,
            "cstb": cstb,
            "cstr": cstr,
        })
    return run_bass_kernel_spmd(nc, in_maps, list(range(N_CORES)), **spmd_kwargs)


def kernel(feat, noise, n_per_graph):
    assert int(n_per_graph) == NPG
    try:
        res = _run(feat, noise)
    except Exception:
        # the axon device occasionally reports a transient unrecoverable
        # state; one retry usually succeeds
        res = _run(feat, noise)
    return np.concatenate([res.results[c]["out"] for c in range(N_CORES)], axis=0)

